# revision 1
# baseline (speedup 1.0000x reference)
"""Per-sample 256-bin histogram -> broadcast [B,256,256], Trainium2 Bass kernel.

Input : x int32 [64, 786432], values in [0, 256)
Output: f32 [64, 256, 256] where out[b, i, j] = count(x[b, :] == i)

Sharding: pure data parallel, 8 rows per core across 8 NeuronCores.

Per-core algorithm (nibble decomposition + paired outer products):
  hist[16h + l] = sum_n onehot16(x_n >> 4)[h] * onehot16(x_n & 15)[l]
  - DVE extracts h/l (int32 shift/and), casts to bf16, builds 16+16 one-hot
    mask sets per tile with is_equal tensor_scalar ops (bf16, 4x mode).
  - Masks are stored [P, 2, 16, T2]: two half-tiles share each matmul.
    PE accumulates [32,32] PSUM outer products where the two diagonal
    [16,16] blocks are the valid per-half histograms (off-diagonal blocks
    are cross-half garbage that is simply never read). 256 elements per
    matmul instruction (~13.5 ns measured) instead of 128.
  - Epilogue per row: 4 partition-scatter DMAs gather the two diagonal
    blocks into [128, 2, 2] columns, one DVE add folds the halves, then
    two broadcast multiplies and two 128KB DMAs write out[r].
  Counts are integer-exact in f32 (max 786432 < 2^24).
"""

import os
import sys

import numpy as np

sys.path.insert(0, "/opt/trn_rl_repo")

B = 64
N = 786432
NCORES = 8
ROWS_PER_CORE = B // NCORES
LEVELS = 256
P = 128

# Tile geometry: T columns per tile -> P*T elements per tile, in two halves.
T = 1024
T2 = T // 2
TILES = N // (P * T)
assert TILES * P * T == N

_cache = {}


def _build_program(rows=None):
    import concourse.bacc as bacc
    from concourse import mybir
    from concourse import tile

    alu = mybir.AluOpType
    dt = mybir.dt

    rows = ROWS_PER_CORE if rows is None else rows
    skip_mm = bool(int(os.environ.get("K_SKIP_MM", "0")))
    skip_masks = bool(int(os.environ.get("K_SKIP_MASKS", "0")))
    skip_epi = bool(int(os.environ.get("K_SKIP_EPI", "0")))

    nc = bacc.Bacc(
        "TRN2",
        target_bir_lowering=False,
        debug=False,
        num_devices=NCORES,
    )
    x_dram = nc.dram_tensor("x", [rows, N], dt.int32, kind="ExternalInput")
    out_dram = nc.dram_tensor(
        "out", [rows, LEVELS, LEVELS], dt.float32, kind="ExternalOutput"
    )

    xv = x_dram.ap().rearrange("r (t p f) -> r t p f", p=P, f=T)
    ov = out_dram.ap()

    import concourse.bass as bass

    use_loop = bool(int(os.environ.get("K_USE_LOOP", "0")))

    with tile.TileContext(nc) as tc:
        with (
            tc.tile_pool(name="xin", bufs=3) as xpool,
            tc.tile_pool(name="hl", bufs=2) as hlpool,
            tc.tile_pool(name="mask", bufs=2) as mpool,
            tc.tile_pool(name="acc", bufs=2, space="PSUM") as ppool,
            tc.tile_pool(name="epi", bufs=2) as epool,
            tc.tile_pool(name="const", bufs=1) as cpool,
        ):
            ones_t = cpool.tile([P, LEVELS], dt.float32)
            nc.vector.memset(ones_t[:], 1.0)

            cm_h = cm_l = None
            if skip_masks:
                cm_h = cpool.tile([P, 2, 16, T2], dt.bfloat16)
                cm_l = cpool.tile([P, 2, 16, T2], dt.bfloat16)
                nc.vector.memset(cm_h[:], 1.0)
                nc.vector.memset(cm_l[:], 1.0)

            def row_body(r):
                dyn = not isinstance(r, int)
                dmae = nc.gpsimd if dyn else nc.sync
                psum_hist = ppool.tile([32, 32], dt.float32, tag="psum_hist")
                for t in range(TILES):
                    xin = xv[bass.ds(r, 1), t] if dyn else xv[r, t]
                    xt = xpool.tile([P, T], dt.int32, tag="xt")
                    # split across DMA queues for bandwidth
                    qs = T // 4
                    for q in range(4):
                        dmae.dma_start(
                            out=xt[:, q * qs : (q + 1) * qs],
                            in_=xin[..., q * qs : (q + 1) * qs],
                        )

                    # h = x >> 4, l = x & 15 (int32; bitwise ops cannot cast)
                    hi = hlpool.tile([P, T], dt.int32, tag="hi")
                    li = hlpool.tile([P, T], dt.int32, tag="li")
                    nc.vector.tensor_scalar(
                        out=hi[:], in0=xt[:], scalar1=4, scalar2=None,
                        op0=alu.logical_shift_right,
                    )
                    nc.vector.tensor_scalar(
                        out=li[:], in0=xt[:], scalar1=15, scalar2=None,
                        op0=alu.bitwise_and,
                    )
                    # cast to bf16 (exact ints 0..15) so masks run in 4x mode
                    hb = hlpool.tile([P, T], dt.bfloat16, tag="hb")
                    lb = hlpool.tile([P, T], dt.bfloat16, tag="lb")
                    nc.vector.tensor_copy(out=hb[:], in_=hi[:])
                    nc.vector.tensor_copy(out=lb[:], in_=li[:])
                    hbv = hb[:].rearrange("p (g f) -> p g f", g=2)
                    lbv = lb[:].rearrange("p (g f) -> p g f", g=2)

                    if skip_masks:
                        hm, lm = cm_h, cm_l
                    else:
                        hm = mpool.tile([P, 2, 16, T2], dt.bfloat16, tag="hm")
                        lm = mpool.tile([P, 2, 16, T2], dt.bfloat16, tag="lm")
                        for a in range(16):
                            nc.vector.tensor_scalar(
                                out=hm[:, :, a, :], in0=hbv, scalar1=float(a),
                                scalar2=None, op0=alu.is_equal,
                            )
                            nc.vector.tensor_scalar(
                                out=lm[:, :, a, :], in0=lbv, scalar1=float(a),
                                scalar2=None, op0=alu.is_equal,
                            )

                    if not skip_mm:
                        for c in range(T2):
                            nc.tensor.matmul(
                                out=psum_hist[:],
                                lhsT=hm[:, :, :, c],
                                rhs=lm[:, :, :, c],
                                start=(t == 0 and c == 0),
                                stop=(t == TILES - 1 and c == T2 - 1),
                            )
                    elif t == 0:
                        nc.tensor.matmul(
                            out=psum_hist[:], lhsT=hm[:, :, :, 0],
                            rhs=lm[:, :, :, 0], start=True, stop=True,
                        )

                # --- epilogue for row r ---
                hist32 = epool.tile([32, 32], dt.float32, tag="hist32")
                nc.vector.tensor_copy(out=hist32[:], in_=psum_hist[:])
                if skip_epi:
                    for half in range(2):
                        bt = epool.tile([P, LEVELS], dt.float32, tag="bt")
                        nc.vector.tensor_copy(out=bt[:], in_=ones_t[:])
                        oout = (ov[bass.ds(r, 1), half * P : (half + 1) * P, :]
                                if dyn else ov[r, half * P : (half + 1) * P, :])
                        dmae.dma_start(out=oout, in_=bt[:])
                    return
                # gather diagonal blocks: histcol2[i, half, g] for i = 16h+l
                histcol2 = epool.tile([P, 2, 2], dt.float32, tag="histcol2")
                nc.sync.dma_start(
                    out=histcol2[:, 0, 0:1], in_=hist32[0:8, 0:16]
                )
                nc.sync.dma_start(
                    out=histcol2[:, 0, 1:2], in_=hist32[16:24, 16:32]
                )
                nc.sync.dma_start(
                    out=histcol2[:, 1, 0:1], in_=hist32[8:16, 0:16]
                )
                nc.sync.dma_start(
                    out=histcol2[:, 1, 1:2], in_=hist32[24:32, 16:32]
                )
                histcol = epool.tile([P, 2], dt.float32, tag="histcol")
                nc.vector.tensor_tensor(
                    out=histcol[:], in0=histcol2[:, :, 0], in1=histcol2[:, :, 1],
                    op=alu.add,
                )

                for half in range(2):
                    bt = epool.tile([P, LEVELS], dt.float32, tag="bt")
                    nc.vector.tensor_scalar(
                        out=bt[:], in0=ones_t[:],
                        scalar1=histcol[:, half : half + 1],
                        scalar2=None, op0=alu.mult,
                    )
                    oout = (ov[bass.ds(r, 1), half * P : (half + 1) * P, :]
                            if dyn else ov[r, half * P : (half + 1) * P, :])
                    dmae.dma_start(out=oout, in_=bt[:])

            if use_loop:
                with tc.For_i(0, rows, 1) as rv:
                    row_body(rv)
            else:
                for r in range(rows):
                    row_body(r)

    nc.compile()
    return nc


def _get_program(rows=None):
    key = ("nc", rows)
    if key not in _cache:
        _cache[key] = _build_program(rows)
    return _cache[key]


def kernel(x: np.ndarray) -> np.ndarray:
    from concourse.bass_utils import run_bass_kernel_spmd

    x = np.ascontiguousarray(np.asarray(x), dtype=np.int32)
    assert x.shape == (B, N), x.shape

    nc = _get_program()
    in_maps = [
        {"x": x[c * ROWS_PER_CORE : (c + 1) * ROWS_PER_CORE]} for c in range(NCORES)
    ]
    res = run_bass_kernel_spmd(nc, in_maps, core_ids=list(range(NCORES)))
    out = np.concatenate([res.results[c]["out"] for c in range(NCORES)], axis=0)
    return out.astype(np.float32)

